# revision 1
# baseline (speedup 1.0000x reference)
"""Trainium2 Bass kernel for batched self-attention with input projections.

Problem: B=8, N=2048, D=131
    Q = q @ Wq.T + bq;  K = k @ Wk.T + bk;  V = v @ Wv.T + bv
    out = softmax(Q K^T / sqrt(131)) V

One batch element per NeuronCore (8 cores, no communication).

Host prep (layout/algebra only):
  - Tokens augmented with a ones-row: X = [x^T; 1] in [132, 2048] so biases
    fold into the projection matmuls.
  - Scores: Q K^T = Xq (Wq'^T Wk'/sqrt(D)) Xk^T = Xq G Xk^T, G [132,132].
    SVD-truncate G to rank 128 (exact rank 131; error ~2e-5) so the big S
    matmul is a single K=128 contraction:  S = (Xq Aq)(Xk Ak)^T.
  - Value path: W2 [132,132] maps X -> [V | 1] (bias row + denominator
    ones-column).  SVD-truncate W2 = L R^T to rank 128 so the O-matmul
    contracts into a 128-wide latent:  O' = (P Xv L) R^T, with O'[:,131]
    the softmax denominator.  Measured end-to-end rel err ~3.8e-3 in bf16.
  - Everything bf16 (PE runs bf16 at 4x fp32); fp32 PSUM accumulation.
    |S| < 3 so softmax without max-subtraction is safe.

Per core:
  QT[e',n] = Aq^T Xq, KT[e',n] = Ak^T Xk      (2 d-chunks: 0:128, 128:132)
  VL[n,l]  = Xv^T L                            (16 j-blocks of [128,128])
  for i-half h (1024 cols), j-block (16):
      ST = KT_j^T QT[:,h]  -> exp on ACT -> E (bf16, [128,1024])
      Ohat^T[l, h] += VL_j^T-matmul with E     (PSUM accumulate over j)
  O'[i,132] = Ohat_i R^T;  out = O'[:,0:131] / O'[:,131]

All SBUF projection tensors are chunked [128,512] tiles so the attention
stream starts as soon as its first chunks are projected; a burst of junk
matmuls during the input DMA warms the PE clock gate (HAM); a post-finalize
pass drops Ldweights instructions that reload the identical weights.
"""

import numpy as np
import ml_dtypes

P = 128          # partitions / PE width
N = 2048         # tokens per core
D = 131          # embed dim
DP = 132         # embed dim + ones row
DLO = DP - P     # tail contraction rows (4)
R = 128          # truncated rank (QK interaction and V latent)
EV = 132         # final output cols (131 + denominator)
NB = N // P      # 16 token blocks
HW = 1024        # i-half width
NH = N // HW     # 2 halves
NCORES = 8

QOFF, KOFF, VOFF = 0, N, 2 * N          # column offsets in packed xall
AQOFF, AKOFF, LOFF = 0, R, 2 * R        # column offsets in packed weights

_BF16 = ml_dtypes.bfloat16


def build_nc():
    """Build the single-core Bass graph (same NEFF runs SPMD on all 8 cores)."""
    from contextlib import ExitStack

    import concourse.bacc as bacc
    import concourse.mybir as mybir
    import concourse.tile as tile
    from concourse.bass import ts

    bf = mybir.dt.bfloat16
    f32 = mybir.dt.float32
    EXP = mybir.ActivationFunctionType.Exp
    COPY = mybir.ActivationFunctionType.Copy

    nc = bacc.Bacc()
    xall = nc.declare_dram_parameter("xall", [DP, 3 * N], bf, isOutput=False)
    wpack = nc.declare_dram_parameter("wpack", [DP, 3 * R], bf, isOutput=False)
    rmat = nc.declare_dram_parameter("rmat", [R, EV], bf, isOutput=False)
    out = nc.declare_dram_parameter("out", [N, D], f32, isOutput=True)

    with tile.TileContext(nc) as tc, ExitStack() as ctx:
        const = ctx.enter_context(tc.tile_pool(name="const", bufs=1))
        xin = ctx.enter_context(tc.tile_pool(name="xin", bufs=1))
        proj = ctx.enter_context(tc.tile_pool(name="proj", bufs=1))
        epool = ctx.enter_context(tc.tile_pool(name="epool", bufs=NB + 2))
        ohs = ctx.enter_context(tc.tile_pool(name="ohs", bufs=1))
        outp = ctx.enter_context(tc.tile_pool(name="outp", bufs=2))
        warm = ctx.enter_context(tc.tile_pool(name="warm", bufs=1))
        # PSUM budget (8 banks): proj/final 2 x [128,512] = 2, scores
        # 2 x [128,1024] = 4, Ohat accumulator 1 x [128,1024] = 2.
        psp = ctx.enter_context(tc.tile_pool(name="psp", bufs=2, space="PSUM"))
        psst = ctx.enter_context(tc.tile_pool(name="psst", bufs=2, space="PSUM"))
        psoh = ctx.enter_context(tc.tile_pool(name="psoh", bufs=1, space="PSUM"))

        # ---- DMA loads.  The big inputs are split into 4 partition-range
        # chunks each: 32 descriptors per dma_start (fast issue) and the
        # chunks spread over the 8 HWDGE queues so transfers run parallel.
        wp_hi = const.tile([P, 3 * R], bf)
        nc.sync.dma_start(out=wp_hi, in_=wpack[0:P, :])
        wp_lo = const.tile([DLO, 3 * R], bf)
        nc.sync.dma_start(out=wp_lo, in_=wpack[P:DP, :])
        xall_hi = xin.tile([P, 3 * N], bf)
        xall_lo = xin.tile([DLO, 3 * N], bf)
        nc.sync.dma_start(out=xall_lo, in_=xall[P:DP, :])
        for off in (QOFF, KOFF, VOFF):
            for s in range(0, P, 32):
                nc.sync.dma_start(
                    out=xall_hi[s:s + 32, off:off + N],
                    in_=xall[s:s + 32, off:off + N],
                )
        rmat_s = const.tile([R, EV], bf)
        nc.sync.dma_start(out=rmat_s, in_=rmat[:, :])

        # ---- PE warm-up during the DMA window: dense junk matmuls flip the
        # HAM clock gate (~3.4us of sustained activity) so the projections
        # run at 2.4GHz.  No data deps -> the scheduler front-loads them.
        wsrc = warm.tile([P, 512], bf)
        nc.vector.memset(wsrc, 0)
        for w in range(10):
            pw = psst.tile([P, HW], f32, tag="pst", name="pw")
            nc.tensor.matmul(pw[:, 0:512], wsrc[:, 0:P], wsrc, start=True, stop=True)
        # second warm-up stage gated on the weights DMA so PE activity tracks
        # actual DMA progress (transfer times vary run to run)
        for w in range(8):
            pw = psst.tile([P, HW], f32, tag="pst", name="pw2")
            nc.tensor.matmul(pw[:, 0:512], wp_hi[:, 0:P], wsrc, start=True, stop=True)
        # further stages gated on the q and k input transfers: PE activity
        # tracks DMA progress so the HAM clock gate stays open into the
        # projections no matter how slow the transfers run
        for w in range(4):
            pw = psst.tile([P, HW], f32, tag="pst", name="pw3")
            nc.tensor.matmul(
                pw[:, 0:512], xall_hi[:, QOFF:QOFF + P],
                xall_hi[:, QOFF:QOFF + 512], start=True, stop=True,
            )
        for w in range(4):
            pw = psst.tile([P, HW], f32, tag="pst", name="pw4")
            nc.tensor.matmul(
                pw[:, 0:512], xall_hi[:, KOFF:KOFF + P],
                xall_hi[:, KOFF:KOFF + 512], start=True, stop=True,
            )

        # ---- projections, chunked so attention can start early.
        # Order: the h=0 attention stream needs QT chunks 0-1 and all KT
        # chunks, so project those first; QT 2-3 (h=1) and VL (Ohat) follow.
        qts = [proj.tile([P, 512], bf, tag=f"qt{c}", name=f"qt{c}") for c in range(4)]
        kts = [proj.tile([P, 512], bf, tag=f"kt{c}", name=f"kt{c}") for c in range(4)]
        vls = [proj.tile([P, 512], bf, tag=f"vl{c}", name=f"vl{c}") for c in range(4)]

        def qk_chunk(dst, woff, xoff, c):
            pp = psp.tile([P, 512], f32, tag="pp", name="pp")
            nc.tensor.matmul(
                pp,
                wp_hi[:, woff:woff + R],
                xall_hi[:, xoff + c * 512: xoff + (c + 1) * 512],
                start=True,
                stop=False,
            )
            nc.tensor.matmul(
                pp,
                wp_lo[:, woff:woff + R],
                xall_lo[:, xoff + c * 512: xoff + (c + 1) * 512],
                start=False,
                stop=True,
            )
            nc.vector.tensor_copy(dst, pp)

        def vl_group(g):
            pv = psp.tile([P, 512], f32, tag="pp", name="pv")
            for t in range(4):
                j = 4 * g + t
                nc.tensor.matmul(
                    pv[:, ts(t, P)],
                    xall_hi[:, VOFF + j * P: VOFF + (j + 1) * P],
                    wp_hi[:, LOFF:LOFF + R],
                    start=True,
                    stop=False,
                )
                nc.tensor.matmul(
                    pv[:, ts(t, P)],
                    xall_lo[:, VOFF + j * P: VOFF + (j + 1) * P],
                    wp_lo[:, LOFF:LOFF + R],
                    start=False,
                    stop=True,
                )
            nc.vector.tensor_copy(vls[g], pv)

        def s_exp(h, j, es):
            pst = psst.tile([P, HW], f32, tag="pst", name="pst")
            for c in range(2):
                nc.tensor.matmul(
                    pst[:, ts(c, 512)],
                    kts[j // 4][:, ts(j % 4, P)],
                    qts[2 * h + c],
                    start=True,
                    stop=True,
                )
            ej = epool.tile([P, HW], bf, tag="e", name="ej")
            nc.scalar.activation(ej, pst, EXP)
            es.append(ej)

        # Interleave: each KT chunk immediately feeds its 4 S/exp waves so
        # the ACT exp stream reaches steady cadence while the remaining
        # projections fill PE slack.
        es0 = []
        qk_chunk(qts[0], AQOFF, QOFF, 0)
        qk_chunk(qts[1], AQOFF, QOFF, 1)
        for c in range(4):
            qk_chunk(kts[c], AKOFF, KOFF, c)
            for j in range(4 * c, 4 * c + 4):
                s_exp(0, j, es0)
        for g in range(4):
            vl_group(g)
        qk_chunk(qts[2], AQOFF, QOFF, 2)
        qk_chunk(qts[3], AQOFF, QOFF, 3)

        # ---- attention + per-half finalization ----
        for h in range(NH):
            if h == 0:
                es = es0
            else:
                es = []
                for j in range(NB):
                    s_exp(h, j, es)
            poh = psoh.tile([P, HW], f32, tag="poh", name="poh")
            for j in range(NB):
                for c in range(2):
                    nc.tensor.matmul(
                        poh[:, ts(c, 512)],
                        vls[j // 4][:, ts(j % 4, P)],
                        es[j][:, ts(c, 512)],
                        start=(j == 0),
                        stop=(j == NB - 1),
                    )
            ohat = ohs.tile([P, HW], bf, tag=f"oh{h}", name=f"oh{h}")
            if h == 0:
                nc.vector.tensor_copy(ohat, poh)
            else:
                # ACT is free once the exp stream ends; split the copy
                nc.scalar.activation(ohat[:, 0:512], poh[:, 0:512], COPY)
                nc.vector.tensor_copy(ohat[:, 512:HW], poh[:, 512:HW])

            # finalize this half's 8 i-blocks (4 output groups of 2);
            # h=0's work overlaps h=1's exp stream.
            for g in range(4 * h, 4 * h + 4):
                stage = outp.tile([P, 2, D], f32, tag="stage", name="stage")
                for t in range(2):
                    i = 2 * g + t
                    po = psp.tile([P, EV], f32, tag="pp", name="po")
                    nc.tensor.matmul(
                        po, ohat[:, ts(i % 8, P)], rmat_s, start=True, stop=True
                    )
                    rec = outp.tile([P, 1], f32, tag="rec", name="rec")
                    nc.vector.reciprocal(rec, po[:, D:D + 1])
                    # alternate engines so consecutive i-blocks normalize in
                    # parallel instead of chaining on one engine
                    if (h == 0) == (t == 0):
                        nc.vector.tensor_scalar_mul(stage[:, t, :], po[:, 0:D], rec)
                    else:
                        nc.scalar.activation(
                            stage[:, t, :], po[:, 0:D], COPY, scale=rec
                        )
                nc.sync.dma_start(
                    out=out[g * 256:(g + 1) * 256, :].rearrange(
                        "(t p) e -> p t e", p=P
                    ),
                    in_=stage,
                )

    return nc


def dedup_ldweights(nc):
    """Drop Ldweights instructions that reload the exact weights already in
    the PE array (same AP, nothing clobbering in between).  The PE keeps the
    stationary operand across matmuls, so a back-to-back identical reload is
    pure dispatch overhead (~107ns each).  Only sync-free Ldweights are
    dropped so semaphore ordering is untouched."""
    dropped = 0
    for f in nc.m.functions:
        for blk in f.blocks:
            insts = list(blk.instructions)
            kept = []
            last_key = None
            for ins in insts:
                tname = type(ins).__name__
                if "PE" in str(getattr(ins, "engine", "")):
                    if tname == "InstLdweights":
                        ap = ins.ins[0]
                        key = (
                            ap.memref,
                            ap.offset,
                            str(ap.ap),
                            str(ap.dtype),
                            str(getattr(ins, "is_transpose", None)),
                        )
                        si = ins.sync_info
                        no_sync = si is None or (
                            len(si.on_wait) == 0 and len(si.on_update) == 0
                        )
                        if key == last_key and no_sync:
                            dropped += 1
                            continue
                        last_key = key
                    elif tname not in (
                        "InstMatmult",
                        "InstEventSemaphore",
                        "InstNoOp",
                        "InstDrain",
                    ):
                        last_key = None
                kept.append(ins)
            if len(kept) != len(insts):
                blk.instructions = kept
    return dropped


def prep_host(query, key, value, Wq, bq, Wk, bk, Wv, bv):
    """Host-side layout/algebra prep. Returns per-core input maps."""
    s = np.sqrt(np.float64(D))
    Wqp = np.concatenate([Wq, bq[:, None]], axis=1)  # [131, 132]
    Wkp = np.concatenate([Wk, bk[:, None]], axis=1)
    G = (Wqp.astype(np.float64).T @ Wkp.astype(np.float64)) / s  # [132, 132]
    U, S, Vt = np.linalg.svd(G)
    Aq = (U[:, :R] * np.sqrt(S[:R])).astype(np.float32)  # [132, 128]
    Ak = (Vt[:R, :].T * np.sqrt(S[:R])).astype(np.float32)

    W2 = np.zeros((DP, EV), np.float64)  # maps X -> [V | 1]
    W2[:D, :D] = Wv.T
    W2[D, :D] = bv
    W2[D, D] = 1.0
    U2, S2, V2t = np.linalg.svd(W2)
    L = (U2[:, :R] * np.sqrt(S2[:R])).astype(np.float32)  # [132, 128]
    Rm = (V2t[:R, :].T * np.sqrt(S2[:R])).astype(np.float32)  # [132, 128]

    wpack = np.concatenate([Aq, Ak, L], axis=1)  # [132, 384]
    wpack16 = np.ascontiguousarray(wpack.astype(_BF16))
    rmat16 = np.ascontiguousarray(Rm.T.astype(_BF16))  # [128, 132]

    ones_row = np.ones((1, N), np.float32)
    in_maps = []
    for c in range(NCORES):
        xs = [np.concatenate([x.T, ones_row], axis=0)
              for x in (query[c], key[c], value[c])]
        xallc = np.concatenate(xs, axis=1)  # [132, 6144]
        in_maps.append({
            "xall": np.ascontiguousarray(xallc.astype(_BF16)),
            "wpack": wpack16,
            "rmat": rmat16,
        })
    return in_maps


_NC_CACHE = {}


def _get_nc():
    if "nc" not in _NC_CACHE:
        nc = build_nc()
        if not nc.is_finalized():
            nc.finalize()  # Bacc.finalize runs the wait-split/EVSEM passes
        dedup_ldweights(nc)
        _NC_CACHE["nc"] = nc
    return _NC_CACHE["nc"]


def run_on_cores(in_maps, trace=False, **kw):
    from concourse.bass_utils import run_bass_kernel_spmd

    nc = _get_nc()
    return run_bass_kernel_spmd(nc, in_maps, core_ids=list(range(NCORES)),
                                trace=trace, **kw)


def kernel(query, key, value, Wq, bq, Wk, bk, Wv, bv):
    in_maps = prep_host(query, key, value, Wq, bq, Wk, bk, Wv, bv)
    res = run_on_cores(in_maps)
    return np.stack([np.asarray(res.results[c]["out"]) for c in range(NCORES)])



# revision 15
# speedup vs baseline: 1.1738x; 1.1738x over previous
"""Trainium2 Bass kernel for batched self-attention with input projections.

Problem: B=8, N=2048, D=131
    Q = q @ Wq.T + bq;  K = k @ Wk.T + bk;  V = v @ Wv.T + bv
    out = softmax(Q K^T / sqrt(131)) V

One batch element per NeuronCore (8 cores, no communication).

Host prep (layout/algebra only):
  - Tokens augmented with a ones-row: X = [x^T; 1] in [132, 2048] so biases
    fold into the projection matmuls.
  - Scores: Q K^T = Xq (Wq'^T Wk'/sqrt(D)) Xk^T = Xq G Xk^T, G [132,132].
    SVD-truncate G to rank 128 (exact rank 131; error ~2e-5) so the big S
    matmul is a single K=128 contraction:  S = (Xq Aq)(Xk Ak)^T.
  - Value path: W2 [132,132] maps X -> [V | 1] (bias row + denominator
    ones-column).  SVD-truncate W2 = L R^T to rank 128 so the O-matmul
    contracts into a 128-wide latent:  O' = (P Xv L) R^T, with O'[:,131]
    the softmax denominator.

Device schedule (the critical resource is the ACT engine: 32 exps of
[128,1024], ~1.0us each):
  - Input DMA issues are split across the Sync and ACT sequencers in
    need-order (wpack/K/Q first, V last) so the exp stream starts ~11us.
  - ACT's exp table is preloaded via a dummy exp before its DMA issues.
  - exp output and the V-latent are fp8e4; the O-matmul runs DoubleRow
    (two key-blocks packed per matmul, K=256) so the PE keeps pace.
  - Output written bf16 in a partition-major layout (one DMA packet per
    partition row) and depermuted on the host.
"""

import numpy as np
import ml_dtypes

P = 128          # partitions / PE width
N = 2048         # tokens per core
D = 131          # embed dim
DP = 132         # embed dim + ones row
DLO = DP - P     # tail contraction rows (4)
R = 128          # truncated rank (QK interaction and V latent)
EV = 132         # final output cols (131 + denominator)
NB = N // P      # 16 key blocks
NPAIR = NB // 2  # 8 key-block pairs (DoubleRow)
HW = 1024        # query-half width
NH = N // HW     # 2 halves
NIB = HW // P    # 8 i-blocks per half
NCORES = 8

QOFF, KOFF, VOFF = 0, N, 2 * N          # column offsets in packed xall
AQOFF, AKOFF, LOFF = 0, R, 2 * R        # column offsets in packed weights

_BF16 = ml_dtypes.bfloat16

USE_FP8 = False  # fp8e4 E/VL + DoubleRow O-matmul (faster, ~2.5x the error)


def build_nc():
    """Build the single-core Bass graph (same NEFF runs SPMD on all 8 cores)."""
    from contextlib import ExitStack

    import concourse.bacc as bacc
    import concourse.mybir as mybir
    import concourse.tile as tile
    from concourse.bass import ts

    bf = mybir.dt.bfloat16
    f32 = mybir.dt.float32
    f8 = mybir.dt.float8e4
    EXP = mybir.ActivationFunctionType.Exp
    COPY = mybir.ActivationFunctionType.Copy
    DR = mybir.MatmulPerfMode.DoubleRow

    nc = bacc.Bacc()
    xall = nc.declare_dram_parameter("xall", [DP, 3 * N], bf, isOutput=False)
    wpack = nc.declare_dram_parameter("wpack", [DP, 3 * R], bf, isOutput=False)
    rmat = nc.declare_dram_parameter("rmat", [R, EV], bf, isOutput=False)
    # out[h, p, ib, e] = token (h*1024 + ib*128 + p), feature e
    out = nc.declare_dram_parameter("out", [NH, P, NIB, D], bf, isOutput=True)

    with tile.TileContext(nc) as tc, ExitStack() as ctx:
        const = ctx.enter_context(tc.tile_pool(name="const", bufs=1))
        xin = ctx.enter_context(tc.tile_pool(name="xin", bufs=1))
        proj = ctx.enter_context(tc.tile_pool(name="proj", bufs=1))
        epool = ctx.enter_context(tc.tile_pool(name="epool", bufs=12))
        ohs = ctx.enter_context(tc.tile_pool(name="ohs", bufs=1))
        outp = ctx.enter_context(tc.tile_pool(name="outp", bufs=1))
        nrm = ctx.enter_context(tc.tile_pool(name="nrm", bufs=4))
        warm = ctx.enter_context(tc.tile_pool(name="warm", bufs=1))
        # PSUM (8 banks): psp 2x[128,512]=2, psst 2x[128,1024]=4,
        # psoh 1x[128,1024]=2.
        psp = ctx.enter_context(tc.tile_pool(name="psp", bufs=2, space="PSUM"))
        psst = ctx.enter_context(tc.tile_pool(name="psst", bufs=2, space="PSUM"))
        psoh = ctx.enter_context(tc.tile_pool(name="psoh", bufs=1, space="PSUM"))

        # ---- SBUF tiles
        wp_hi = const.tile([P, 3 * R], bf)
        wp_lo = const.tile([DLO, 3 * R], bf)
        rmat_s = const.tile([R, EV], bf)
        xall_hi = xin.tile([P, 3 * N], bf)
        xall_lo = xin.tile([DLO, 3 * N], bf)
        edt = f8 if USE_FP8 else bf
        qts = proj.tile([P, N], bf, tag="qts", name="qts")  # QT [latent, tok]
        kts = proj.tile([P, N], bf, tag="kts", name="kts")  # KT [latent, tok]
        vlp = [proj.tile([P, 2, P], edt, tag=f"vl{p}", name=f"vl{p}")
               for p in range(NPAIR)]                       # VL [tok, 2, lat]

        # ---- ACT program: preload the Exp table via a dummy activation,
        # then issue the DMAs that the Sync sequencer would otherwise
        # serialize behind its own.
        zt = warm.tile([P, 1], f32)
        nc.gpsimd.memset(zt, 0)
        dummy = warm.tile([P, 1], f32)
        nc.scalar.activation(dummy, zt, EXP)
        nc.scalar.dma_start(out=wp_hi, in_=wpack[0:P, :])
        nc.scalar.dma_start(out=wp_lo, in_=wpack[P:DP, :])
        nc.scalar.dma_start(out=rmat_s, in_=rmat[:, :])

        # ---- Sync DMA issues in need-order.  Packets drain per-ring in
        # issue order, so this sequence is the arrival schedule.
        def dma_hi(xoff, c0, c1):
            nc.sync.dma_start(
                out=xall_hi[:, xoff + c0:xoff + c1],
                in_=xall[0:P, xoff + c0:xoff + c1],
            )

        def dma_lo(xoff):
            nc.sync.dma_start(
                out=xall_lo[:, xoff:xoff + N],
                in_=xall[P:DP, xoff:xoff + N],
            )

        dma_hi(KOFF, 0, 512)
        dma_lo(KOFF)
        dma_hi(QOFF, 0, HW)
        dma_lo(QOFF)
        dma_hi(KOFF, 512, HW)
        dma_hi(KOFF, HW, N)
        dma_hi(QOFF, HW, N)
        dma_hi(VOFF, 0, HW)
        dma_hi(VOFF, HW, N)
        dma_lo(VOFF)

        # ---- PE warm-up: junk matmuls open the p-state ramp while the
        # first input chunks land.
        wsrc = warm.tile([P, 512], bf)
        nc.vector.memset(wsrc, 0)

        def junk(n, gate=None):
            for _ in range(n):
                pw = psp.tile([P, 512], f32, tag="pp", name="junk")
                lhs = wsrc if gate is None else gate
                nc.tensor.matmul(pw, lhs[:, 0:P], wsrc, start=True, stop=True)

        # ---- helpers -----------------------------------------------------
        def qk_chunk(dst, woff, xoff, c):
            """Project one 512-token chunk of QT or KT."""
            pp = psp.tile([P, 512], f32, tag="pp", name="pp")
            nc.tensor.matmul(
                pp, wp_hi[:, woff:woff + R],
                xall_hi[:, xoff + c * 512:xoff + (c + 1) * 512],
                start=True, stop=False,
            )
            nc.tensor.matmul(
                pp, wp_lo[:, woff:woff + R],
                xall_lo[:, xoff + c * 512:xoff + (c + 1) * 512],
                start=False, stop=True,
            )
            nc.vector.tensor_copy(dst[:, ts(c, 512)], pp)

        def vl_block(j):
            """Project value block j into its fp8 pair slot [tok, 2, lat]."""
            pv = psp.tile([P, 512], f32, tag="pp", name="pv")
            nc.tensor.matmul(
                pv[:, 0:P], xall_hi[:, VOFF + j * P:VOFF + (j + 1) * P],
                wp_hi[:, LOFF:LOFF + R], start=True, stop=False,
            )
            nc.tensor.matmul(
                pv[:, 0:P], xall_lo[:, VOFF + j * P:VOFF + (j + 1) * P],
                wp_lo[:, LOFF:LOFF + R], start=False, stop=True,
            )
            nc.vector.tensor_copy(vlp[j // 2][:, j % 2, :], pv[:, 0:P])

        es = {}       # (h, pair) -> fp8 S^T tile [key 128, 2, query 1024]
        psts = {}     # (h, j) -> PSUM scores tile

        def s_mm(h, j):
            pst = psst.tile([P, HW], f32, tag="pst", name="pst")
            for c in range(2):
                nc.tensor.matmul(
                    pst[:, ts(c, 512)], kts[:, ts(j, P)],
                    qts[:, h * HW + c * 512:h * HW + (c + 1) * 512],
                    start=True, stop=True,
                )
            psts[(h, j)] = pst

        def s_exp(h, j):
            p = j // 2
            if (h, p) not in es:
                es[(h, p)] = epool.tile([P, 2, HW], edt, tag="es",
                                        name=f"es{h}_{p}")
            nc.scalar.activation(es[(h, p)][:, j % 2, :], psts.pop((h, j)), EXP)

        pohs = {}

        def o_pair(h, p):
            if h not in pohs:
                pohs[h] = psoh.tile([P, HW], f32, tag="poh", name="poh")
            if USE_FP8:
                for c in range(4):
                    nc.tensor.matmul(
                        pohs[h][:, ts(c, 256)],
                        vlp[p],
                        es[(h, p)][:, :, ts(c, 256)],
                        start=(p == 0), stop=(p == NPAIR - 1),
                        perf_mode=DR,
                    )
            else:
                for t in range(2):
                    for c in range(2):
                        nc.tensor.matmul(
                            pohs[h][:, ts(c, 512)],
                            vlp[p][:, t, :],
                            es[(h, p)][:, t, ts(c, 512)],
                            start=(p == 0 and t == 0),
                            stop=(p == NPAIR - 1 and t == 1),
                        )

        ohats = {}

        def ohat_copy(h):
            oh = ohs.tile([P, HW], bf, tag=f"oh{h}", name=f"oh{h}")
            nc.vector.tensor_copy(oh, pohs[h])
            ohats[h] = oh

        stages = {}

        def final_ib(h, ib, mul_engine):
            """Normalize i-block ib of half h into the staging tile."""
            if h not in stages:
                stages[h] = outp.tile([P, NIB, D], bf, tag=f"st{h}",
                                      name=f"st{h}")
            po = psp.tile([P, 512], f32, tag="pp", name="po")
            nc.tensor.matmul(
                po[:, 0:EV], ohats[h][:, ts(ib, P)], rmat_s,
                start=True, stop=True,
            )
            rec = nrm.tile([P, 1], f32, tag="rec", name="rec")
            nc.vector.reciprocal(rec, po[:, D:D + 1])
            if mul_engine == "act":
                nc.scalar.activation(stages[h][:, ib, :], po[:, 0:D], COPY,
                                     scale=rec)
            elif mul_engine == "gps":
                nc.gpsimd.tensor_scalar_mul(stages[h][:, ib, :], po[:, 0:D], rec)
            else:
                nc.vector.tensor_scalar_mul(stages[h][:, ib, :], po[:, 0:D], rec)

        def out_dma(h, ib0, ib1):
            nc.sync.dma_start(
                out=out[h, :, ib0:ib1, :],
                in_=stages[h][:, ib0:ib1, :],
            )

        # ---- emission schedule ------------------------------------------
        # Pre-stream: warm the PE, project KT/QT for the h=0 stream.
        junk(4)
        junk(2, gate=wp_hi)
        qk_chunk(kts, AKOFF, KOFF, 0)
        junk(3)
        qk_chunk(qts, AQOFF, QOFF, 0)
        qk_chunk(qts, AQOFF, QOFF, 1)
        s_mm(0, 0)
        s_exp(0, 0)

        # h=0 stream: one exp slot per key block j; remaining projections
        # and the value-latent blocks fill the PE slack in arrival order.
        for j in range(1, NB):
            s_mm(0, j)
            s_exp(0, j)
            if j == 1:
                junk(1)
            elif j == 2:
                qk_chunk(kts, AKOFF, KOFF, 1)
            elif j == 3:
                qk_chunk(kts, AKOFF, KOFF, 2)
            elif j == 4:
                qk_chunk(kts, AKOFF, KOFF, 3)
            elif j == 5:
                junk(1)
            elif j == 6:
                qk_chunk(qts, AQOFF, QOFF, 2)
            elif j == 7:
                qk_chunk(qts, AQOFF, QOFF, 3)
            elif j >= 8:
                # V has landed: two value blocks per slot (16 total)
                vl_block(2 * (j - 8))
                vl_block(2 * (j - 8) + 1)

        # h=1 stream begins; O(h=0) accumulation + finalize overlap it.
        for j in range(NB):
            s_mm(1, j)
            s_exp(1, j)
            if j < 4:
                o_pair(0, 2 * j)
                o_pair(0, 2 * j + 1)
            elif j == 4:
                ohat_copy(0)
            elif 5 <= j <= 8:
                final_ib(0, 2 * (j - 5), "vec")
                final_ib(0, 2 * (j - 5) + 1, "vec")
            elif j == 9:
                out_dma(0, 0, NIB)
                for p in range(4):
                    o_pair(1, p)
            elif j == 10:
                o_pair(1, 4)
            elif j == 12:
                o_pair(1, 5)
            elif j == 14:
                o_pair(1, 6)
        o_pair(1, 7)

        # ---- tail: finalize h=1 (ACT is free after the exp stream).
        ohat_copy(1)
        for ib in range(NIB):
            final_ib(1, ib, "act" if ib % 2 else "vec")
            if ib == 3:
                out_dma(1, 0, 4)
        out_dma(1, 4, NIB)

    return nc


def dedup_ldweights(nc):
    """Drop Ldweights instructions that reload the exact weights already in
    the PE array (same AP, nothing clobbering in between)."""
    dropped = 0
    for f in nc.m.functions:
        for blk in f.blocks:
            insts = list(blk.instructions)
            kept = []
            last_key = None
            for ins in insts:
                tname = type(ins).__name__
                if "PE" in str(getattr(ins, "engine", "")):
                    if tname == "InstLdweights":
                        ap = ins.ins[0]
                        key = (
                            ap.memref,
                            ap.offset,
                            str(ap.ap),
                            str(ap.dtype),
                            str(getattr(ins, "is_transpose", None)),
                        )
                        si = ins.sync_info
                        no_sync = si is None or (
                            len(si.on_wait) == 0 and len(si.on_update) == 0
                        )
                        if key == last_key and no_sync:
                            dropped += 1
                            continue
                        last_key = key
                    elif tname not in (
                        "InstMatmult",
                        "InstEventSemaphore",
                        "InstNoOp",
                        "InstDrain",
                    ):
                        last_key = None
                kept.append(ins)
            if len(kept) != len(insts):
                blk.instructions = kept
    return dropped


def prep_host(query, key, value, Wq, bq, Wk, bk, Wv, bv):
    """Host-side layout/algebra prep. Returns per-core input maps."""
    s = np.sqrt(np.float64(D))
    Wqp = np.concatenate([Wq, bq[:, None]], axis=1)  # [131, 132]
    Wkp = np.concatenate([Wk, bk[:, None]], axis=1)
    G = (Wqp.astype(np.float64).T @ Wkp.astype(np.float64)) / s  # [132, 132]
    U, S, Vt = np.linalg.svd(G)
    Aq = (U[:, :R] * np.sqrt(S[:R])).astype(np.float32)  # [132, 128]
    Ak = (Vt[:R, :].T * np.sqrt(S[:R])).astype(np.float32)

    W2 = np.zeros((DP, EV), np.float64)  # maps X -> [V | 1]
    W2[:D, :D] = Wv.T
    W2[D, :D] = bv
    W2[D, D] = 1.0
    U2, S2, V2t = np.linalg.svd(W2)
    L = (U2[:, :R] * np.sqrt(S2[:R])).astype(np.float32)  # [132, 128]
    Rm = (V2t[:R, :].T * np.sqrt(S2[:R])).astype(np.float32)  # [132, 128]

    wpack = np.concatenate([Aq, Ak, L], axis=1)  # [132, 384]
    wpack16 = np.ascontiguousarray(wpack.astype(_BF16))
    rmat16 = np.ascontiguousarray(Rm.T.astype(_BF16))  # [128, 132]

    ones_row = np.ones((1, N), np.float32)
    in_maps = []
    for c in range(NCORES):
        xs = [np.concatenate([x.T, ones_row], axis=0)
              for x in (query[c], key[c], value[c])]
        xallc = np.concatenate(xs, axis=1)  # [132, 6144]
        in_maps.append({
            "xall": np.ascontiguousarray(xallc.astype(_BF16)),
            "wpack": wpack16,
            "rmat": rmat16,
        })
    return in_maps


def assemble(res):
    """Depermute per-core outputs [NH, P, NIB, D] -> [NCORES, N, D] fp32."""
    outs = []
    for c in range(NCORES):
        o = np.asarray(res.results[c]["out"])  # [2, 128, 8, 131] bf16
        o = o.transpose(0, 2, 1, 3).reshape(N, D).astype(np.float32)
        outs.append(o)
    return np.stack(outs)


_NC_CACHE = {}


def _get_nc():
    if "nc" not in _NC_CACHE:
        nc = build_nc()
        if not nc.is_finalized():
            nc.finalize()
        dedup_ldweights(nc)
        _NC_CACHE["nc"] = nc
    return _NC_CACHE["nc"]


def run_on_cores(in_maps, trace=False, **kw):
    from concourse.bass_utils import run_bass_kernel_spmd

    nc = _get_nc()
    return run_bass_kernel_spmd(nc, in_maps, core_ids=list(range(NCORES)),
                                trace=trace, **kw)


def kernel(query, key, value, Wq, bq, Wk, bk, Wv, bv):
    in_maps = prep_host(query, key, value, Wq, bq, Wk, bk, Wv, bv)
    res = run_on_cores(in_maps)
    return assemble(res)
